# revision 14
# baseline (speedup 1.0000x reference)
"""Trainium2 Bass kernel for localized 3x3-window multi-head attention.

Problem: B=8, N=4096 (64x64 grid), DIM=512, 8 heads x 64 dim, KSIZE=3.
  qkv = x @ w_qkv; per-head localized attention over zero-padded 3x3
  spatial neighborhood; out = attn_out @ w_out + b_out.

Sharding: data-parallel over batch - core i computes batch i (8 cores).

v4 (2-tile-pair batching + engine rebalance over v3):
  - attention stage processes 256-token pairs: halves per-op fixed
    overhead on DVE/ACT (measured ~240ns/op) for P9, softmax, AV ops.
  - AX expansion (attn weights broadcast over head_dim) done as ONE
    int32-bitcast broadcast copy per pair: bf16 head-pairs ride in one
    i32 lane, halving expansion element count.  V / w_out columns are
    host-permuted to (head-pair, d, head-in-pair) order to match.
  - b_out folded into MM2 as a K=1 ones-row matmul; output copied
    PSUM->SBUF on ACT (frees a DVE op per tile).
  - AV add-tree alternates DVE / GpSimd to offload the busiest engine.
  - dots ones-matmuls ordered b-outer to batch identical LDWEIGHTS.
"""

import numpy as np
import ml_dtypes

bf16 = ml_dtypes.bfloat16

B, N, DIM = 8, 4096, 512
HEADS, HEAD_DIM, K9 = 8, 64, 9
GRID = 64          # 64x64 spatial grid
PAD = 65           # max |token shift| = 64+1
NT = N // 128      # 32 token tiles per core
NC4 = N // 512     # 8 512-token chunks per core
VROWS = N + 2 * PAD
HALO = 2 * PAD + 512   # haloed token window per chunk (642)

_CACHE = {}

# opt-in profiling knobs (test.py sets these; harness leaves defaults)
TRACE = False
TRACE_DIR = None
LAST_RESULTS = None
# AX expansion engine: "act" | "vector" (set before first kernel() call)
AX_ENGINE = "act"


def _build():
    import concourse.bass as bass
    import concourse.mybir as mybir
    import concourse.tile as tile
    from concourse import bacc
    from concourse.bass import ts
    from concourse.masks import make_identity

    fp32 = mybir.dt.float32
    b16 = mybir.dt.bfloat16
    i32 = mybir.dt.int32
    Copy = mybir.ActivationFunctionType.Copy

    nc = bacc.Bacc("TRN2", target_bir_lowering=False, debug=False)

    xT = nc.dram_tensor("xT", [DIM, N], b16, kind="ExternalInput")
    wq = nc.dram_tensor("wq", [DIM, 3 * DIM], b16, kind="ExternalInput")
    wo = nc.dram_tensor("wo", [DIM, DIM], b16, kind="ExternalInput")
    bbr = nc.dram_tensor("bbr", [1, DIM], b16, kind="ExternalInput")
    # wrap mask / wrap count, [kk, h] layout
    wmask = nc.dram_tensor("wmask", [128, K9 * HEADS], fp32, kind="ExternalInput")
    nw = nc.dram_tensor("nw", [128, 1], fp32, kind="ExternalInput")
    out = nc.dram_tensor("out", [N, DIM], fp32, kind="ExternalOutput")

    with tile.TileContext(nc) as tc:
        with (
            tc.tile_pool(name="const", bufs=1) as const,
            tc.tile_pool(name="dram", bufs=1, space="DRAM") as dpool,
            tc.tile_pool(name="qkt", bufs=2) as qktpool,
            tc.tile_pool(name="vs", bufs=3) as vspool,
            tc.tile_pool(name="v9", bufs=2) as v9pool,
            tc.tile_pool(name="attn", bufs=2) as apool,
            tc.tile_pool(name="ax", bufs=1) as axpool,
            tc.tile_pool(name="prod", bufs=3) as ppool,
            tc.tile_pool(name="outp", bufs=2) as opool,
            tc.tile_pool(name="psqk", bufs=2, space="PSUM") as psqk,
            tc.tile_pool(name="psd", bufs=1, space="PSUM") as psd,
            tc.tile_pool(name="psdt", bufs=1, space="PSUM") as psdt,
            tc.tile_pool(name="pstp", bufs=1, space="PSUM") as pstp,
            tc.tile_pool(name="psm2", bufs=1, space="PSUM") as psm2,
        ):
            # ---- constants ----
            xT_sb = [const.tile([128, N], b16, name=f"xT{c}") for c in range(4)]
            wq_sb = [const.tile([128, 3 * DIM], b16, name=f"wq{c}") for c in range(4)]
            wo_sb = [const.tile([128, DIM], b16, name=f"wo{c}") for c in range(4)]
            for c in range(4):
                nc.sync.dma_start(out=xT_sb[c], in_=xT[ts(c, 128), :])
                nc.sync.dma_start(out=wq_sb[c], in_=wq[ts(c, 128), :])
                nc.sync.dma_start(out=wo_sb[c], in_=wo[ts(c, 128), :])
            bbr_sb = const.tile([1, DIM], b16, name="bbr")
            nc.sync.dma_start(out=bbr_sb, in_=bbr[:, :])
            onesrow = const.tile([1, 128], b16, name="onesrow")
            nc.vector.memset(onesrow, 1.0)
            wm_sb = const.tile([128, K9 * HEADS], fp32, name="wm")
            nc.sync.dma_start(out=wm_sb, in_=wmask[:, :])
            nw_sb = const.tile([128, 1], fp32, name="nw")
            nc.sync.dma_start(out=nw_sb, in_=nw[:, :])
            ident = const.tile([128, 128], b16, name="ident")
            make_identity(nc, ident)
            zero_sb = const.tile([128, DIM], b16, name="zero")
            nc.vector.memset(zero_sb, 0.0)
            # per-block ones weights: block b maps its two 64-partition head
            # segments to psum rows 2b / 2b+1 (other columns zero, so PSUM
            # accumulation stacks the four blocks into one [8, n] tile)
            onesb = []
            for b in range(4):
                ob = const.tile([128, 8], b16, name=f"onesw{b}")
                nc.vector.memset(ob, 0.0)
                for h2 in range(2):
                    nc.vector.memset(
                        ob[64 * h2:64 * (h2 + 1), 2 * b + h2:2 * b + h2 + 1],
                        1.0)
                onesb.append(ob)

            # ---- V DRAM scratch with zero pad rows ----
            vdr = dpool.tile([VROWS, DIM], b16, name="vscratch")
            nc.sync.dma_start(out=vdr[0:PAD, :], in_=zero_sb[0:PAD, :])
            nc.sync.dma_start(out=vdr[PAD + N:VROWS, :], in_=zero_sb[0:PAD, :])

            qT_tiles = {}   # chunk -> [4 blocks] of [128, 512]
            kT_tiles = {}   # chunk -> [4 blocks] of [128, HALO]
            v9_tiles = {}   # pair t0 -> [128, 2, 3, 3, DIM]

            def mm1(C):
                """Chunk C (512 tokens): qT,kT head-major; V token-major."""
                qTb = [qktpool.tile([128, 512], b16, tag=f"qT{b}",
                                    name=f"qT{b}_{C}") for b in range(4)]
                kTb = [qktpool.tile([128, HALO], b16, tag=f"kT{b}",
                                    name=f"kT{b}_{C}") for b in range(4)]
                qT_tiles[C] = qTb
                kT_tiles[C] = kTb
                for b in range(4):
                    for which in range(2):  # 0 = q, 1 = k
                        ps = psqk.tile([128, 512], fp32, tag="qk")
                        mcol = which * DIM + b * 128
                        for c in range(4):
                            nc.tensor.matmul(
                                ps, lhsT=wq_sb[c][:, mcol:mcol + 128],
                                rhs=xT_sb[c][:, ts(C, 512)],
                                start=(c == 0), stop=(c == 3))
                        if which == 0:
                            # fold attention scale into qT
                            nc.scalar.activation(qTb[b], ps, Copy, scale=0.125)
                        else:
                            nc.scalar.activation(kTb[b][:, PAD:PAD + 512], ps,
                                                 Copy)
                            if C > 0:
                                # my first 65 tokens are C-1's right halo
                                nc.scalar.activation(
                                    kT_tiles[C - 1][b][:, PAD + 512:HALO],
                                    ps[:, 0:PAD], Copy)
                                # C-1's last 65 tokens are my left halo
                                nc.vector.tensor_copy(
                                    kTb[b][:, 0:PAD],
                                    kT_tiles[C - 1][b][:, 512:512 + PAD])
                            else:
                                nc.vector.memset(kTb[b][:, 0:PAD], 0.0)
                            if C == NC4 - 1:
                                nc.vector.memset(kTb[b][:, PAD + 512:HALO], 0.0)
                # V token-major, per 128-token tile
                for tt in range(4):
                    t = 4 * C + tt
                    psv_t = psqk.tile([128, DIM], fp32, tag="qk")
                    for c in range(4):
                        nc.tensor.matmul(
                            psv_t, lhsT=xT_sb[c][:, ts(t, 128)],
                            rhs=wq_sb[c][:, 2 * DIM:3 * DIM],
                            start=(c == 0), stop=(c == 3))
                    vt = vspool.tile([128, DIM], b16, tag="vst")
                    nc.scalar.activation(vt, psv_t, Copy)
                    nc.gpsimd.dma_start(
                        out=vdr[PAD + t * 128: PAD + (t + 1) * 128, :], in_=vt)

            def prefetch_v_pair(t0):
                v9t = v9pool.tile([128, 2, 3, 3, DIM], b16, tag="v9")
                for ti in range(2):
                    t = t0 + ti
                    for di in range(3):
                        base = (t * 128 + 64 * di) * DIM
                        src = bass.AP(tensor=vdr.tensor,
                                      offset=vdr.offset + base,
                                      ap=[[DIM, 128], [DIM, 3], [1, DIM]])
                        nc.sync.dma_start(out=v9t[:, ti, di, :, :], in_=src)
                v9_tiles[t0] = v9t

            def attn_pair(t0):
                C, tt0 = t0 // 4, t0 % 4
                qTb, kTb = qT_tiles[C], kT_tiles[C]
                v9t = v9_tiles.pop(t0)
                # all-9-shift products per 2-head block over BOTH tiles
                P9s = []
                for b in range(4):
                    P9 = apool.tile([128, 3, 3, 256], b16, tag=f"P9_{b}")
                    qsl = qTb[b][:, tt0 * 128:tt0 * 128 + 256]
                    qin = qsl.unsqueeze(1).unsqueeze(1).broadcast_to(
                        (128, 3, 3, 256))
                    kbase = kTb[b][:, 0:1]
                    kin = bass.AP(
                        tensor=kbase.tensor, offset=kbase.offset + tt0 * 128,
                        ap=[list(kbase.ap[0]), [64, 3], [1, 3], [1, 256]])
                    nc.vector.tensor_mul(P9, qin, kin)
                    P9s.append(P9)
                # ones-block matmuls contract d, PSUM-accumulating the four
                # blocks; b-outer so each LDWEIGHTS serves 6 matmuls.
                # Both tiles of the pair share one psum tile: ti=0 on
                # partitions 0-7, ti=1 on 32-39 (col-tiled matmuls).
                dps = [psd.tile([40, 3 * 128], fp32, tag=f"d{s}",
                                name=f"dps{s}_{t0}") for s in range(3)]
                for b in range(4):
                    for ti in range(2):
                        for s in range(3):
                            rhs = P9s[b][:, s, :, ti * 128:(ti + 1) * 128]
                            nc.tensor.matmul(
                                dps[s][32 * ti:32 * ti + 8, :],
                                lhsT=onesb[b], rhs=rhs,
                                start=(b == 0), stop=(b == 3))
                # dots -> SBUF bf16, tiny PE transposes to token-major
                dsb = apool.tile([8, 2, 3, 384], b16, tag="dsb")
                for ti in range(2):
                    for s in range(3):
                        nc.scalar.activation(dsb[:, ti, s, :],
                                             dps[s][32 * ti:32 * ti + 8, :],
                                             Copy)
                dsb9 = dsb.rearrange("p t s (c x) -> p t (s c) x", c=3)
                dtok = psdt.tile([128, 2, K9 * 8], b16, tag="dtok")
                for ti in range(2):
                    for kk in range(K9):
                        nc.tensor.transpose(dtok[:, ti, ts(kk, 8)],
                                            dsb9[:, ti, kk, :],
                                            ident[0:8, 0:8])
                # token-major softmax over the pair, [kk, h] axis order
                E2 = apool.tile([128, 2, K9 * HEADS], fp32, tag="E2")
                nc.scalar.activation(E2, dtok,
                                     mybir.ActivationFunctionType.Exp)
                A2 = apool.tile([128, 2, K9, HEADS], fp32, tag="A2")
                nc.vector.tensor_mul(
                    A2.rearrange("p t k h -> p t (k h)"), E2,
                    wm_sb.unsqueeze(1).broadcast_to((128, 2, K9 * HEADS)))
                Z2 = apool.tile([128, 2, HEADS], fp32, tag="Z2")
                nc.vector.tensor_reduce(
                    Z2, A2.rearrange("p t k h -> p t h k"),
                    axis=mybir.AxisListType.X, op=mybir.AluOpType.add)
                Z2f = Z2.rearrange("p t h -> p (t h)")
                nc.vector.tensor_scalar_add(Z2f, Z2f, nw_sb)
                Zr2 = apool.tile([128, 2, HEADS], fp32, tag="Zr2")
                nc.vector.reciprocal(Zr2.rearrange("p t h -> p (t h)"), Z2f)
                Ab2 = apool.tile([128, 2, K9, HEADS], b16, tag="Ab2")
                nc.vector.tensor_mul(
                    Ab2, A2,
                    Zr2.unsqueeze(2).broadcast_to((128, 2, K9, HEADS)))

                # AX expansion. V / w_out are host-permuted so column
                # 128*hp + 2*d + j holds (head 2*hp+j, dim d): bf16
                # head-pair values sit adjacent, one i32 lane.
                AX2 = axpool.tile([128, 2, K9, DIM], b16, tag="AX2")
                if AX_ENGINE == "vector":
                    # one bit-exact i32 broadcast on DVE
                    ax_dst = AX2.bitcast(i32).rearrange(
                        "p t k (hp d) -> p t (k hp) d", hp=4)
                    ax_src = Ab2.bitcast(i32).rearrange(
                        "p t k hp -> p t (k hp)").unsqueeze(3).broadcast_to(
                        (128, 2, K9 * 4, HEAD_DIM))
                    nc.vector.tensor_copy(ax_dst, ax_src)
                else:
                    # two bf16 broadcasts on ACT (one per head-in-pair j);
                    # i32 path is not bit-safe through the ACT datapath
                    AX2v = AX2.rearrange(
                        "p t k (hp d j) -> p (t k) hp d j", hp=4, j=2)
                    Ab2v = Ab2.rearrange(
                        "p t k (hp j) -> p (t k) hp j", j=2)
                    for j in range(2):
                        nc.scalar.activation(
                            AX2v[:, :, :, :, j],
                            Ab2v[:, :, :, j].unsqueeze(3).broadcast_to(
                                (128, 2 * K9, 4, HEAD_DIM)), Copy)

                # AV: 9 muls (DVE) + serial add-chain alternating DVE/GpSimd
                acc = ppool.tile([128, 2, DIM], b16, tag="acc")
                Pm0 = ppool.tile([128, 2, DIM], b16, tag="Pm")
                nc.vector.tensor_mul(acc, AX2[:, :, 0, :],
                                     v9t[:, :, 0, 0, :])
                nc.vector.tensor_mul(Pm0, AX2[:, :, 1, :],
                                     v9t[:, :, 0, 1, :])
                nc.vector.tensor_add(acc, acc, Pm0)
                for kk in range(2, K9):
                    di, dj = kk // 3, kk % 3
                    Pm = ppool.tile([128, 2, DIM], b16, tag="Pm")
                    nc.vector.tensor_mul(Pm, AX2[:, :, kk, :],
                                         v9t[:, :, di, dj, :])
                    if kk in (4, 8):
                        nc.gpsimd.tensor_add(acc, acc, Pm)
                    else:
                        nc.vector.tensor_add(acc, acc, Pm)

                # MM2 with b_out folded in as a K=1 ones-row matmul
                for ti in range(2):
                    tp = pstp.tile([128, DIM], b16, tag="tp")
                    for c in range(4):
                        nc.tensor.transpose(tp[:, ts(c, 128)],
                                            acc[:, ti, ts(c, 128)], ident)
                    lhsT = opool.tile([128, DIM], b16, tag="lhsT")
                    nc.scalar.activation(lhsT, tp, Copy)
                    ps2 = psm2.tile([128, DIM], fp32, tag="mm2")
                    nc.tensor.matmul(ps2, lhsT=onesrow, rhs=bbr_sb,
                                     start=True, stop=False)
                    for c in range(4):
                        nc.tensor.matmul(ps2, lhsT=lhsT[:, ts(c, 128)],
                                         rhs=wo_sb[c], start=False,
                                         stop=(c == 3))
                    otp = opool.tile([128, DIM], fp32, tag="otp")
                    nc.scalar.activation(otp, ps2, Copy)
                    nc.gpsimd.dma_start(out=out[ts(t0 + ti, 128), :],
                                        in_=otp)

            # chunk-level software pipeline: mm1(C) runs one chunk ahead of
            # attention (kT right-halo of C-1 is filled during mm1(C))
            for C in range(NC4 + 1):
                if C < NC4:
                    mm1(C)
                if C >= 1:
                    base = 4 * (C - 1)
                    prefetch_v_pair(base)
                    prefetch_v_pair(base + 2)
                    attn_pair(base)
                    attn_pair(base + 2)

    nc.compile()
    return nc


def _perm():
    # V / w_out column permutation: (h, d) -> (head-pair, d, head-in-pair)
    # so bf16 head-pair values are adjacent (one int32 lane) in AX / V / AV
    p = np.zeros(DIM, dtype=np.int64)
    for hp in range(4):
        for d in range(HEAD_DIM):
            for j in range(2):
                p[128 * hp + 2 * d + j] = 64 * (2 * hp + j) + d
    return p


def _wrap_mask():
    # wmask[p, kk, h] = 0 where the dj=+-1 neighbor wraps around a grid row
    m = np.ones((128, K9, HEADS), dtype=np.float32)
    for p in range(128):
        j = p % GRID
        for kk in range(K9):
            dj = kk % 3 - 1
            if (j == 0 and dj == -1) or (j == GRID - 1 and dj == 1):
                m[p, kk, :] = 0.0
    return m.reshape(128, K9 * HEADS)


def kernel(x, w_qkv, w_out, b_out, h_img=64, w_img=64):
    from concourse.bass_utils import run_bass_kernel_spmd

    assert int(h_img) == GRID and int(w_img) == GRID
    if "nc" not in _CACHE:
        _CACHE["nc"] = _build()
    nc = _CACHE["nc"]

    perm = _perm()
    wqh = np.ascontiguousarray(w_qkv.astype(bf16))
    wqh[:, 2 * DIM:3 * DIM] = wqh[:, 2 * DIM:3 * DIM][:, perm]
    woh = np.ascontiguousarray(w_out.astype(bf16)[perm, :])
    bbr = b_out.astype(bf16).reshape(1, DIM)
    wm = _wrap_mask()
    # 3 wrapped window entries (one per di) at each grid-row edge
    nw = np.zeros((128, 1), dtype=np.float32)
    nw[np.arange(128) % GRID == 0] = 3.0
    nw[np.arange(128) % GRID == GRID - 1] = 3.0
    in_maps = []
    for i in range(B):
        xTi = np.ascontiguousarray(x[i].T.astype(bf16))
        in_maps.append(dict(xT=xTi, wq=wqh, wo=woh, bbr=bbr, wmask=wm, nw=nw))

    kw = {}
    if TRACE:
        kw = dict(trace=True, tmpdir=TRACE_DIR)
    res = run_bass_kernel_spmd(nc, in_maps, core_ids=list(range(B)), **kw)
    global LAST_RESULTS
    LAST_RESULTS = res
    return np.stack([r["out"] for r in res.results], axis=0)
